# revision 5
# baseline (speedup 1.0000x reference)
"""3x3 stride-2 VALID avg-pool over (8, 64, 512, 512) fp32 on 8 trn2 cores.

Sharding: data-parallel over batch — core i handles x[i] (64 planes of
512x512, contiguous 64 MiB slab). No communication.

Per-core dataflow (planes processed in pairs to halve PE weight-loads):
  1. DMA one plane (1 MiB, contiguous) into SBUF as [128p, 4r, 512w]
     (row h = 4p + r).
  2. DVE W-pool via strided views:  rp[p,r,j] = x[h,2j]+x[h,2j+1]+x[h,2j+2]
     (2 tensor_add ops per plane over [128, 4, 255]).
  3. PE H-pool as a sparse pooling-matrix matmul, two planes packed into
     one moving operand (N = 2*255 = 510 <= 512 fp32 limit): for each
     output-row chunk mc, accumulate over r:
        psum[m, (cc,j)] += mt[:, mc, r, :].T @ rp[:, r, (cc,j)]
     where mt[k, mc, r, m] = 1 iff input row (4k+r) is in the 3-row window
     of output row (mc*128+m).
  4. ScalarE copy PSUM -> SBUF with scale 1/9.
  5. Batched DMA out (4 planes per store pair, 2 stores per group).
"""

import sys

sys.path.insert(0, "/opt/trn_rl_repo")

import numpy as np

from concourse import bacc, bass, mybir, tile
from concourse.bass_utils import run_bass_kernel_spmd

P = 128
B, C, H, W = 8, 64, 512, 512
KS, ST = 3, 2
HO = (H - KS) // ST + 1  # 255
WO = (W - KS) // ST + 1  # 255
CPC = C  # planes per core (one batch image per core)
GROUP = 4  # planes per output-DMA batch
N_CORES = 8

_F32 = mybir.dt.float32


def _pool_matrices() -> np.ndarray:
    """mt[k, mc, r, m] = 1 iff row h=4k+r feeds output row i=mc*128+m."""
    mt = np.zeros((P, 2, 4, P), np.float32)
    k = np.arange(P)[:, None, None, None]
    mc = np.arange(2)[None, :, None, None]
    r = np.arange(4)[None, None, :, None]
    m = np.arange(P)[None, None, None, :]
    h = 4 * k + r
    i = mc * P + m
    mt[(i < HO) & (2 * i <= h) & (h <= 2 * i + 2)] = 1.0
    return mt


def _build_nc(repeat: int = 1) -> bass.Bass:
    nc = bacc.Bacc(None)
    x = nc.declare_dram_parameter("x", [CPC, H, W], _F32, isOutput=False)
    mt = nc.declare_dram_parameter("mt", [P, 2, 4, P], _F32, isOutput=False)
    out = nc.declare_dram_parameter("out", [CPC, HO, WO], _F32, isOutput=True)

    with tile.TileContext(nc) as tc:
        with (
            tc.tile_pool(name="const", bufs=1) as constp,
            tc.tile_pool(name="xin", bufs=8) as xp,
            tc.tile_pool(name="rp", bufs=4) as rpp,
            tc.tile_pool(name="ob", bufs=3) as obp,
            tc.tile_pool(name="ps", bufs=8, space="PSUM") as psp,
        ):
            mt_sb = constp.tile([P, 2, 4, P], _F32)
            nc.sync.dma_start(out=mt_sb[:], in_=mt[:])

            def body():
                for g in range(CPC // GROUP):
                    ob = obp.tile([P, 2, GROUP, WO], _F32)  # [p, chunk, cc, j]
                    for cc in range(GROUP):
                        c = g * GROUP + cc
                        xt = xp.tile([P, 4, W], _F32)
                        nc.sync.dma_start(
                            out=xt[:],
                            in_=x[c].rearrange("(p r) w -> p r w", p=P),
                        )
                        rp = rpp.tile([P, 4, WO], _F32)
                        nc.vector.tensor_add(
                            rp[:],
                            xt[:, :, 0 : 2 * WO : 2],
                            xt[:, :, 1 : 2 * WO + 1 : 2],
                        )
                        nc.vector.tensor_add(
                            rp[:], rp[:], xt[:, :, 2 : 2 * WO + 2 : 2]
                        )
                        for mc in range(2):
                            pst = psp.tile([P, WO], _F32)
                            for r in range(4):
                                nc.tensor.matmul(
                                    pst[:],
                                    mt_sb[:, mc, r, :],
                                    rp[:, r, :],
                                    start=(r == 0),
                                    stop=(r == 3),
                                )
                            nc.scalar.mul(
                                ob[:, mc, cc, :],
                                pst[:],
                                1.0 / 9.0,
                            )
                    og = out[g * GROUP : (g + 1) * GROUP]  # [GROUP, HO, WO]
                    nc.sync.dma_start(
                        out=og[:, 0:P, :].transpose([1, 0, 2]),
                        in_=ob[:, 0, :, :],
                    )
                    nc.sync.dma_start(
                        out=og[:, P:HO, :].transpose([1, 0, 2]),
                        in_=ob[0 : HO - P, 1, :, :],
                    )

            if repeat == 1:
                body()
            else:
                with tc.For_i(0, repeat, 1):
                    body()
    nc.compile()
    return nc


_NC_CACHE: dict = {}


def _get_nc(repeat: int = 1):
    if repeat not in _NC_CACHE:
        _NC_CACHE[repeat] = _build_nc(repeat)
    return _NC_CACHE[repeat]


def kernel(x: np.ndarray, **_unused) -> np.ndarray:
    assert x.shape == (B, C, H, W), x.shape
    x = np.ascontiguousarray(np.asarray(x, dtype=np.float32))
    mt = _pool_matrices()
    in_maps = [{"x": x[i], "mt": mt} for i in range(N_CORES)]
    res = run_bass_kernel_spmd(_get_nc(), in_maps, list(range(N_CORES)))
    return np.stack([res.results[i]["out"] for i in range(N_CORES)], axis=0)
